# revision 117
# baseline (speedup 1.0000x reference)
"""ALiBi causal attention layer on 8 TRN2 NeuronCores.

Sharding: data parallel on batch (B=2) x tensor parallel on heads (16 -> 4
groups of 4).  Core c = 4*b + g computes, for batch element b, the STRIDED
head set {g, 4+g, 8+g, 12+g} end to end: QKV projections (column-sharded),
causal ALiBi attention, and the row-sharded output projection.  The host
sums the 4 partial outputs per batch element (the tensor-parallel
all-reduce) and adds the output bias.  The striding makes head slot j hold
global heads {4j..4j+3} on every core, so each slot's ALiBi slope range is
uniform and the SPMD-shared graph can window steep slots' attention: slot 0
(slopes >= 0.25) looks back only 56 positions, slot 1 (>= 0.0625) 224 --
skipped k-tiles contribute < 1e-11 to the softmax.

Device kernel (matmuls in bf16/f32r, fp32 PSUM accum):
  - x arrives host-transposed: xt [1024, 2048].  Projection biases never
    touch the device: bv folds into the host-side output bias (softmax
    rows sum to 1), bk's score contribution is constant per query column
    (softmax-invariant, dropped), and bq's surviving rank-1 term
    bq.(Wk x_k) rides a third ALiBi aug row (slots 0,1) / the per-k ACT
    bias table (slots 2,3) -- zeros when bq == 0.
  - K^T for steep slots 0,1 in per-head [128, 2048] f32r tiles: head data
    at its native partition parity, 3 aug rows (k, s8, 8*bqk) paired with
    Q rows (s8, -q, 1), remaining rows zeroed.  S^T = K_aug^T.T @ Q_aug,
    exp() on ACT with scale=1/8 (max-free softmax: scores bounded).
  - Shallow slots 2,3 get per-slot [128, T] bf16 K^T tiles at their
    native parity with the other half zeroed (the packed Q tile's
    other-slot rows multiply zeros); their ALiBi + bq term ride the
    exp's per-partition ACT bias (the per-q part cancels in the softmax).
  - Causality: k-tiles above the diagonal are skipped; diagonal tiles get
    -3000 on masked entries via a 128-col (tt=3: 256-col) staircase
    matmul accumulated pre-exp, so the exp underflows to 0.  Each slot's
    window also trims S/exp/PV columns beyond k_end + W.
  - V carries a ones column per head (den cols zero in the weights, a
    strided SBUF copy writes the ones), so PV yields O^T plus the softmax
    denominators.  All 4 heads' denominators stage at partitions
    {0,32,64,96} of one tile (legal matmul contraction bases): ONE
    reciprocal + ONE f32r cast per q-block serve the 4 broadcast matmuls
    (tile_position row-strips).  The divide writes ot rows directly --
    the DVE *can* remap partition blocks (out 64:128 from in 0:64).
  - Engine choreography: warm-up matmuls + a dummy exp run at t=0 under
    the initial DMA wait (PE p-state ramp + ACT table load off the
    critical path); every tensor loads with 1-2 DMA descriptors from
    host-interleaved [128, n, cols] panels; each head's trailing PVs lag
    the exp by 2 k-tiles and flush inside the NEXT emission site; each
    q-block's output projection is deferred into the NEXT block and
    interleaved round-robin with its projections (PSUM-evacuation lag
    drains under the dense proj groups) and with the pass_b chain; y
    evacuations alternate scalar/vector; PSUM po pool holds all 4 heads
    so PV never couples to pass_b.
"""
import math

import ml_dtypes
import numpy as np

BF = ml_dtypes.bfloat16

import concourse.bass as bass
import concourse.tile as tile
from concourse import mybir, bacc
from concourse.bass_utils import run_bass_kernel_spmd

F32 = mybir.dt.float32
F32R = mybir.dt.float32r
BF16 = mybir.dt.bfloat16

B, T, C, H = 2, 2048, 1024, 16
D = C // H            # 64 head dim
NCORES = 8
HG = 4                # heads per core
CG = HG * D           # 256 channels per core
VW = HG * (D + 1)     # 260: V with a ones column per head
QB = 512              # q block width
KTW = 128             # k tile width
NQB = T // QB         # 4
NKT = T // KTW        # 16
NCH = C // 128        # 8 contraction chunks


def _slopes(n):
    def p2(m):
        start = 2 ** (-(2 ** -(math.log2(m) - 3)))
        return [start * start**i for i in range(m)]
    if math.log2(n).is_integer():
        return p2(n)
    c = 2 ** math.floor(math.log2(n))
    return p2(c) + _slopes(2 * c)[0::2][: n - c]


def _build():
    nc = bacc.Bacc()
    # host pre-interleaves every matrix into [128, n*cols] panels (chunk c
    # of the contraction dim at columns [cols*c, cols*(c+1))) so each
    # tensor loads with a single contiguous DMA descriptor.
    xt = nc.declare_dram_parameter("xt", [128, NCH, T], BF16, isOutput=False)
    wq = nc.declare_dram_parameter("wq", [128, NCH, CG], BF16, isOutput=False)
    wk = nc.declare_dram_parameter("wk", [128, NCH, CG], BF16, isOutput=False)
    wv = nc.declare_dram_parameter("wv", [128, NCH, VW], BF16, isOutput=False)
    wo = nc.declare_dram_parameter("wo", [128, 2, C], BF16, isOutput=False)
    hka = nc.declare_dram_parameter("hka", [2, 3, T], F32R, isOutput=False)
    hqa = nc.declare_dram_parameter("hqa", [2, 3, T], F32R, isOutput=False)
    stair = nc.declare_dram_parameter("stair", [128, 256], BF16, isOutput=False)
    ident = nc.declare_dram_parameter("ident", [128, 128], BF16, isOutput=False)
    hbias = nc.declare_dram_parameter("hbias", [128, 128], F32, isOutput=False)
    y = nc.declare_dram_parameter("y", [T, C], BF16, isOutput=True)

    EXP = mybir.ActivationFunctionType.Exp
    CPY = mybir.ActivationFunctionType.Copy

    with tile.TileContext(nc) as tc, \
         nc.allow_low_precision(reason="fp32r/bf16 compute"):
        with tc.tile_pool(name="const", bufs=1) as cp, \
             tc.tile_pool(name="xtp", bufs=3) as xtp, \
             tc.tile_pool(name="qap", bufs=8) as qap, \
             tc.tile_pool(name="otp", bufs=4) as otp, \
             tc.tile_pool(name="ptp", bufs=10) as ptp, \
             tc.tile_pool(name="yp", bufs=2) as ypool, \
             tc.tile_pool(name="misc", bufs=2) as mp, \
             tc.tile_pool(name="ps", bufs=4, space="PSUM") as psp, \
             tc.tile_pool(name="po", bufs=4, space="PSUM") as pop:

            # ---- t=0: PE p-state warm-up + ACT table load, under the
            # initial DMA wait.  No data deps, so the scheduler runs these
            # immediately; ~3.4us of matmul activity un-throttles the PE
            # clock before the first real projection matmul issues.
            wtile = cp.tile([128, QB], BF16, tag="warm")
            nc.gpsimd.memset(wtile[:], 0.25)
            wps = psp.tile([128, QB], F32, tag="ps", name="warm_ps")
            for i in range(22):
                nc.tensor.matmul(wps[:], wtile[:, 0:128], wtile[:],
                                 start=True, stop=True, skip_group_check=True)
            wrd = mp.tile([1, 16], F32, tag="wrd")
            nc.vector.tensor_copy(wrd[:], wps[0:1, 0:16])
            scr = cp.tile([1, 16], F32, tag="scr")
            nc.scalar.activation(scr[:], wtile[0:1, 0:16], EXP,
                                 bias=0.0, scale=1.0)

            # ---- constants: weights, aug rows ----
            # DMA descriptor generation (~0.5us each) is spread across the
            # sync / scalar / gpsimd queues so the first projection's
            # inputs (wq + xt block 0) land as early as possible.
            wq_big = cp.tile([128, NCH, CG], BF16, tag="wqb")
            wk_big = cp.tile([128, NCH, CG], BF16, tag="wkb")
            wv_big = cp.tile([128, NCH, VW], BF16, tag="wvb")
            wo_big = cp.tile([128, 2, C], BF16, tag="wob")
            wq_sb = [wq_big[:, c, :] for c in range(NCH)]
            wk_sb = [wk_big[:, c, :] for c in range(NCH)]
            wv_sb = [wv_big[:, c, :] for c in range(NCH)]
            wo_sb = [wo_big[:, m, :] for m in range(2)]
            ones_fr = cp.tile([128, 128], F32R, tag="ones_fr")
            ones32 = cp.tile([128, 128], F32, tag="ones32")
            nc.vector.memset(ones32[:], 1.0)
            nc.vector.tensor_copy(ones_fr[:], ones32[:])
            zf = cp.tile([128, QB], F32, tag="zf")
            nc.vector.memset(zf[:], 0.0)
            vones = cp.tile([128, 4], F32, tag="vones")
            nc.vector.memset(vones[:], 1.0)
            # first-needed tensors split in two so the leading projection
            # matmuls start after half the transfer
            nc.scalar.dma_start(wq_big[:, 0:4, :], wq[:, 0:4, :])
            nc.scalar.dma_start(wq_big[:, 4:8, :], wq[:, 4:8, :])
            xta0 = xtp.tile([128, NCH, QB], BF16, tag="xt", name="xta0")
            for qtr in range(4):
                nc.sync.dma_start(xta0[:, 2 * qtr:2 * qtr + 2, :],
                                  xt[:, 2 * qtr:2 * qtr + 2, 0:QB])
            xts0 = [xta0[:, c, :] for c in range(NCH)]

            # causal-mask staircase: stair[p, f] = -3000 where f - 128 < p.
            # Accumulating I.T @ stair into the masked 128 (tt=3: 256)
            # columns of a diagonal S tile drives k > q scores to -3000
            # pre-exp, so the exp underflows to 0.
            stair_sb = cp.tile([128, 256], BF16, tag="stair")
            ident_sb = cp.tile([128, 128], BF16, tag="ident")
            hb_sb = cp.tile([128, 128], F32, tag="hb")
            nc.gpsimd.dma_start(stair_sb[:], stair[:])
            nc.gpsimd.dma_start(ident_sb[:], ident[:])
            nc.gpsimd.dma_start(hb_sb[:], hbias[:])

            # Slots 0,1 (steep ALiBi slopes): per-head K^T tiles with the
            # rank-3 aug-row ALiBi (+ bq rank-1 term).  Even head: data
            # rows 0:64, aug rows 64:67, zeros 67:128.  Odd head: aug 0:3,
            # zeros 3:64, data 64:128.  K aug = (k, s8, 8*bqk).
            # Slots 2,3 (shallow slopes): per-slot [128, T] bf16 K^T tiles
            # at the slot's native parity (slot2 rows 0:64, slot3 rows
            # 64:128) with the other half zeroed -- the packed Q tile's
            # other-slot rows multiply zeros.  Their ALiBi rides the exp
            # as a per-partition ACT bias (the per-q part cancels in the
            # softmax).
            ka2 = cp.tile([128, T], BF16, tag="ka2")
            ka3 = cp.tile([128, T], BF16, tag="ka3")
            ka = [cp.tile([128, T], F32R, tag=f"ka{h}", name=f"ka{h}") for h in range(2)]
            # ka zero-fills ride the startup-idle DVE (the scalar queue
            # must stay clear: it feeds the first exps); wv/wo descriptor
            # generation trails wq/xt0/wk so the startup-critical
            # transfers get the HBM bandwidth first.
            nc.scalar.dma_start(wk_big[:], wk[:])
            for h in range(2):
                arow = 64 if h % 2 == 0 else 0
                for blk in range(NQB):
                    sl = slice(QB * blk, QB * (blk + 1))
                    nc.scalar.mul(ka[h][arow:arow + 64, sl],
                                  zf[arow:arow + 64, :], 0.0)
                nc.gpsimd.dma_start(ka[h][arow:arow + 3, :], hka[h])
            nc.gpsimd.memset(ka2[64:128, :], 0.0)
            nc.gpsimd.memset(ka3[0:64, :], 0.0)
            nc.scalar.dma_start(wv_big[:], wv[:])
            nc.scalar.dma_start(wo_big[:], wo[:])

            v_sb = [cp.tile([128, VW], F32R, tag=f"v{t}", name=f"v{t}") for t in range(NKT)]

            # deferred final-PV + finish-head closures: flushed after the
            # next emission site has queued ready PE work, so the in-order
            # PE queue never parks on the tail exp of a head.
            carry = []

            def drain_carry():
                while carry:
                    carry.pop(0)()

            # ---- fused, software-pipelined per-block loop ----
            def fetch_x(qb):
                tsl = slice(QB * qb, QB * (qb + 1))
                xta = xtp.tile([128, NCH, QB], BF16, tag="xt",
                               name=f"xta{qb}")
                nc.sync.dma_start(xta[:], xt[:, :, tsl])
                return [xta[:, c, :] for c in range(NCH)]

            def proj(qb, xts, out_qa):
                """QKV projections for t-block qb (a generator: yields
                after each matmul group so the caller can interleave other
                emission between groups).  Appends the Q tiles to out_qa."""
                tsl = slice(QB * qb, QB * (qb + 1))

                qa_t = out_qa
                for h in range(2):
                    qat = qap.tile([128, QB], F32R, tag="qa",
                                   name=f"qa{qb}_{h}")
                    arow = 64 if h % 2 == 0 else 0
                    nc.vector.tensor_copy(qat[arow:arow + 64, :],
                                          zf[arow:arow + 64, :])
                    nc.scalar.dma_start(qat[arow:arow + 3, :],
                                        hqa[h][:, tsl])
                    qa_t.append(qat)
                q23 = qap.tile([128, QB], BF16, tag="q23",
                               name=f"q23_{qb}")
                qa_t.append(q23)

                for wi, (wsb, is_q) in enumerate(((wq_sb, True),
                                                  (wk_sb, False))):
                    for m in range(2):
                        ps = psp.tile([128, QB], F32, tag="ps")
                        for c in range(NCH):
                            nc.tensor.matmul(
                                ps[:], wsb[c][:, 128 * m:128 * (m + 1)],
                                xts[c][:], start=(c == 0), stop=(c == 7),
                                skip_group_check=True)
                        if m == 1:
                            # packed pair: slot2 rows 0:64, slot3 rows
                            # 64:128, exactly the proj PSUM layout
                            if is_q:
                                nc.vector.tensor_copy(q23[:], ps[:])
                            else:
                                nc.vector.tensor_copy(ka2[0:64, tsl],
                                                      ps[0:64, :])
                                nc.vector.tensor_copy(ka3[64:128, tsl],
                                                      ps[64:128, :])
                        else:
                            for j in range(2):
                                h = j
                                rows = slice(64 * j, 64 * j + 64)
                                if is_q:
                                    nc.vector.tensor_copy(qa_t[h][rows, :],
                                                          ps[rows, :])
                                else:
                                    nc.vector.tensor_copy(ka[h][rows, tsl],
                                                          ps[rows, :])
                        yield

                for tt in range(4):
                    kt = 4 * qb + tt
                    psv = psp.tile([128, QB], F32, tag="ps")
                    for c in range(NCH):
                        nc.tensor.matmul(
                            psv[:, 0:VW],
                            xts[c][:, 128 * tt:128 * (tt + 1)], wv_sb[c][:],
                            start=(c == 0), stop=(c == 7),
                            skip_group_check=True)
                    nc.vector.tensor_copy(v_sb[kt][:], psv[:, 0:VW])
                    # per-head softmax-denominator ones columns: one
                    # stride-65 copy covers all four
                    nc.vector.tensor_copy(v_sb[kt][:, D:VW:D + 1],
                                          vones[:])
                    yield

            qa_next = []
            for _ in proj(0, xts0, qa_next):
                pass
            pending_out = None
            for qb in range(NQB):
                qa_t = qa_next
                if qb + 1 < NQB:
                    xts_next = fetch_x(qb + 1)
                # attention for this q-block.  Pass A per head is the
                # PE-heavy S/exp/PV chain; pass B (recip -> broadcast
                # -> divide) for head h is emitted after head h+1's pass A
                # so the broadcast matmul never sits at the front of the PE
                # queue waiting on the DVE reciprocal.
                po_t = {}
                ot_t = [otp.tile([128, QB], BF16, tag="ot",
                                 name=f"ot_{qb}_{c}") for c in range(2)]

                # ALiBi windows per head slot: with the strided head
                # assignment, slot j holds global heads {4j..4j+3}; a tile
                # whose every (k, q) pair has slope*(k-q) <= -14 contributes
                # < 1e-4 relative attention mass (well under the 2e-2 rel-err
                # budget).  W_j = 14 / min-slope-in-slot.
                WIN = (56.0, 224.0, 897.0, 1e9)

                # Diagonal k-tile tt (tt = kt - 4*qb) only matters for q
                # columns >= 128*tt, so trim its S/exp/PV to [C_tt, 512).
                # tt=3 keeps 256 cols (f32r needs a >=256 moving dim); its
                # extra cols [256,384) are fully masked by the staircase.
                # The slot's ALiBi window trims the upper end too: columns
                # beyond k_end + W contribute < e^-14 attention mass.
                TRIM = ((0, QB), (128, 384), (256, 256), (256, 256))

                def trim(slot, kt):
                    tt = kt - 4 * qb
                    c0 = TRIM[tt][0] if tt >= 0 else 0
                    wmax = TRIM[tt][1] if tt >= 0 else QB
                    kend = 128 * (kt + 1) - 512 * qb  # block-relative
                    need = kend + WIN[slot] - c0
                    w = min(wmax, max(256, -(-int(need) // 128) * 128))
                    return c0, w

                den4 = mp.tile([128, QB], F32, tag="den4", bufs=2,
                               name=f"den4_{qb}")
                rc4 = mp.tile([128, QB], F32R, tag="rc4", bufs=2,
                              name=f"rc4_{qb}")

                # keep every den4 partition defined so the full-tile
                # reciprocal never reads uninitialized SBUF
                nc.gpsimd.memset(den4[:], 1.0)

                def finish_head(h, po):
                    # head h's denominator to partition 32h of the shared
                    # tile (a legal matmul contraction base); one
                    # reciprocal + one f32r cast then serve all 4 heads.
                    nc.vector.tensor_copy(den4[32 * h:32 * h + 1, :],
                                          po[D:D + 1, :])
                    po_t[h] = po

                def finish_all(tag=0):
                    # full-tile reciprocal/cast: DVE cost is free-dim
                    # dominated; rows whose head isn't finished yet hold
                    # the 1.0 memset (or a stale den) and are either never
                    # read or recomputed by a later call.
                    rc32 = mp.tile([128, QB], F32, tag="rc32", bufs=2,
                                   name=f"rc32_{qb}_{tag}")
                    nc.vector.reciprocal_approx_fast(rc32[:], den4[:])
                    nc.vector.tensor_copy(rc4[:], rc32[:])

                def pass_a(h):
                    # diagonal tiles go first so tile tt=0 opens the full
                    # [0,512) PV accumulation region and the head's tail is
                    # short-latency.  PV lags the S/exp chain by one k-tile
                    # so the PE never sits waiting on the ACT exp.
                    full = [kt for kt in range(4 * qb)
                            if 128 * kt > QB * qb - WIN[h] - 127]
                    kts = list(range(4 * qb, 4 * qb + 4)) + full
                    po = pop.tile([D + 1, QB], F32, tag="po",
                                  name=f"po_{qb}_{h}")
                    npv = [0]

                    def pv(pkt, pc0, pw, ppt, last):
                        nc.tensor.matmul(
                            po[:, pc0:pc0 + pw],
                            v_sb[pkt][:, 65 * h:65 * (h + 1)],
                            ppt[:, 0:pw], start=(npv[0] == 0), stop=last,
                            skip_group_check=True)
                        npv[0] += 1

                    pendq = []
                    for i, kt in enumerate(kts):
                        tt = kt - 4 * qb
                        c0, w = trim(h, kt)
                        pss = psp.tile([128, QB], F32, tag="ps")
                        nc.tensor.matmul(
                            pss[:, 0:w], ka[h][:, 128 * kt:128 * (kt + 1)],
                            qa_t[h][:, c0:c0 + w], start=True, stop=(tt < 0),
                            skip_group_check=True)
                        if tt >= 0:
                            # masked (k > q) entries get -3000 pre-exp; only
                            # the 128 cols crossing the diagonal (tt=3: the
                            # 256 cols at/below it) need the staircase
                            soff, sw = (128, 128) if tt < 3 else (0, 256)
                            nc.tensor.matmul(
                                pss[:, 0:sw], ident_sb[:],
                                stair_sb[:, soff:soff + sw], start=False,
                                stop=True, skip_group_check=True)
                        pt = ptp.tile([128, QB], F32R, tag="pt")
                        nc.scalar.activation(pt[:, 0:w], pss[:, 0:w], EXP,
                                             bias=0.0, scale=0.125)
                        if i == 1:
                            drain_carry()
                        # PV lags the exp by two k-tiles: ~640ns of S work
                        # separates each PV from its exp's completion
                        pendq.append((kt, c0, w, pt))
                        if len(pendq) > 4:
                            pv(*pendq.pop(0), last=False)

                    def fin(pendq=pendq):
                        while len(pendq) > 1:
                            pv(*pendq.pop(0), last=False)
                        pv(*pendq.pop(0), last=True)
                        finish_head(h, po)
                    carry.append(fin)

                def pass_a23():
                    # slots 2,3 share one packed K/Q tile; common k-tiles
                    # issue as two concurrent row-tiled S matmuls.  PV lags
                    # by one k-tile so the PE never waits on the exp.
                    full2 = [kt for kt in range(4 * qb)
                             if 128 * kt > QB * qb - WIN[2] - 127]
                    kts = list(range(4 * qb, 4 * qb + 4)) + list(range(4 * qb))
                    po2 = pop.tile([D + 1, QB], F32, tag="po",
                                   name=f"po_{qb}_2")
                    po3 = pop.tile([D + 1, QB], F32, tag="po",
                                   name=f"po_{qb}_3")
                    n2 = 4 + len(full2)
                    n3 = len(kts)
                    i2 = [0]
                    i3 = [0]

                    def pv_flush(pend):
                        pkt, pc0, pw2, pw3, pt2, pt3 = pend
                        if pt2 is not None:
                            nc.tensor.matmul(
                                po2[:, pc0:pc0 + pw2],
                                v_sb[pkt][:, 65 * 2:65 * 3], pt2[:, 0:pw2],
                                start=(i2[0] == 0), stop=(i2[0] == n2 - 1),
                                skip_group_check=True)
                            i2[0] += 1
                        nc.tensor.matmul(
                            po3[:, pc0:pc0 + pw3],
                            v_sb[pkt][:, 65 * 3:65 * 4], pt3[:, 0:pw3],
                            start=(i3[0] == 0), stop=(i3[0] == n3 - 1),
                            skip_group_check=True)
                        i3[0] += 1

                    pendq = []
                    for i, kt in enumerate(kts):
                        tt = kt - 4 * qb
                        c0, w2 = trim(2, kt)
                        c0, w3 = trim(3, kt)
                        ktsl = slice(128 * kt, 128 * (kt + 1))
                        has2 = tt >= 0 or kt in full2
                        if has2:
                            pss2 = psp.tile([128, QB], F32, tag="ps")
                            nc.tensor.matmul(
                                pss2[:, 0:w2], ka2[:, ktsl],
                                qa_t[2][:, c0:c0 + w2], start=True,
                                stop=(tt < 0), skip_group_check=True)
                        pss3 = psp.tile([128, QB], F32, tag="ps")
                        nc.tensor.matmul(
                            pss3[:, 0:w3], ka3[:, ktsl],
                            qa_t[2][:, c0:c0 + w3], start=True,
                            stop=(tt < 0), skip_group_check=True)
                        if tt >= 0:
                            soff, sw = (128, 128) if tt < 3 else (0, 256)
                            nc.tensor.matmul(
                                pss2[:, 0:sw], ident_sb[:],
                                stair_sb[:, soff:soff + sw], start=False,
                                stop=True, skip_group_check=True)
                            nc.tensor.matmul(
                                pss3[:, 0:sw], ident_sb[:],
                                stair_sb[:, soff:soff + sw], start=False,
                                stop=True, skip_group_check=True)
                        pt2 = None
                        if has2:
                            bcol = 16 * qb + kt
                            pt2 = ptp.tile([128, QB], F32R, tag="pt")
                            nc.scalar.activation(
                                pt2[:, 0:w2], pss2[:, 0:w2], EXP,
                                bias=hb_sb[:, bcol:bcol + 1], scale=0.125)
                        bcol = 64 + 16 * qb + kt
                        pt3 = ptp.tile([128, QB], F32R, tag="pt")
                        nc.scalar.activation(
                            pt3[:, 0:w3], pss3[:, 0:w3], EXP,
                            bias=hb_sb[:, bcol:bcol + 1], scale=0.125)
                        if i == 1:
                            drain_carry()
                        pendq.append((kt, c0, w2, w3, pt2, pt3))
                        if len(pendq) > 4:
                            pv_flush(pendq.pop(0))

                    def fin(pendq=pendq):
                        while pendq:
                            pv_flush(pendq.pop(0))
                        finish_head(3, po3)
                        finish_head(2, po2)
                    carry.append(fin)

                def pass_b(h):
                    po = po_t.pop(h)
                    pb = psp.tile([D, QB], F32, tag="ps",
                                  name=f"pb_{qb}_{h}")
                    nc.tensor.matmul(pb[:], ones_fr[32 * h:32 * h + 1, 0:D],
                                     rc4[32 * h:32 * h + 1, :],
                                     start=True, stop=True,
                                     skip_group_check=True,
                                     tile_position=(32 * h, 0))
                    bc = mp.tile([D, QB], F32, tag="bc", bufs=4,
                                 name=f"bc_{qb}_{h}")
                    nc.vector.tensor_copy(bc[:], pb[:])
                    pair = ot_t[h // 2]
                    if h % 2 == 0:
                        nc.vector.tensor_tensor(pair[0:D, :], po[0:D, :],
                                                bc[:],
                                                op=mybir.AluOpType.mult)
                    else:
                        # odd head's O^T lands at partitions 0:64; DVE
                        # cannot shift multi-partition blocks, so divide
                        # into a temp then DMA it into rows 64:128
                        nc.vector.tensor_tensor(pair[D:2 * D, :],
                                                po[0:D, :], bc[:],
                                                op=mybir.AluOpType.mult)

                def emit_outproj(oqb, ot_pair, dual=False):
                    """Generator: yields after each half-tile so the
                    caller can interleave projection groups between the
                    PSUM-evacuation-paced output-projection groups.  With
                    dual=True each evacuation is split across the scalar
                    AND vector engines (for the final block, which has no
                    filler to hide the evacuation lag behind)."""
                    for tt in range(4):
                        t = 4 * oqb + tt
                        fsl = slice(128 * tt, 128 * (tt + 1))
                        ysb = ypool.tile([128, C], BF16, tag="y",
                                         name=f"y_{oqb}_{tt}")
                        for half in range(2):
                            hsl = slice(QB * half, QB * (half + 1))
                            py = psp.tile([128, QB], F32, tag="ps")
                            for c in (1, 0):
                                nc.tensor.matmul(
                                    py[:], ot_pair[c][:, fsl],
                                    wo_sb[c][:, hsl],
                                    start=(c == 1), stop=(c == 0),
                                    skip_group_check=True)
                            # alternate the PSUM evacuation between the
                            # scalar and vector engines: neither engine
                            # alone can keep up with the PE here
                            if dual:
                                h0 = slice(QB * half, QB * half + 256)
                                h1 = slice(QB * half + 256, QB * (half + 1))
                                nc.scalar.activation(ysb[:, h0],
                                                     py[:, 0:256], CPY)
                                nc.vector.tensor_copy(ysb[:, h1],
                                                      py[:, 256:QB])
                            elif (2 * tt + half) % 2 == 0:
                                nc.scalar.activation(ysb[:, hsl], py[:], CPY)
                            else:
                                nc.vector.tensor_copy(ysb[:, hsl], py[:])
                            # the final block's writebacks alternate queues
                            # so their descriptor generation parallelizes
                            # (it's the kernel's tail)
                            dq = (nc.sync if dual and half == 0 else
                                  nc.gpsimd)
                            dq.dma_start(y[128 * t:128 * (t + 1), hsl],
                                         ysb[:, hsl])
                            yield

                # Slots 2,3 (largest k-tile count) first.  All four heads'
                # denominators land in den4, then one reciprocal serves the
                # four pass_b's.  The PREVIOUS q-block's output projection
                # and the NEXT block's projections are emitted under the
                # pass_b chains: the scheduler fills every PE stall with
                # that ready work.
                pass_a23()
                pass_a(1)
                last = qb + 1 == NQB
                pass_a(0)
                # interleave three streams over the rest of the block:
                # the previous block's output projection (PSUM-evacuation
                # paced), the next block's input projections (dense PE
                # filler), and this block's pass_b chain (DVE-latency
                # bound) -- plus the deferred attention-tail PVs, drained
                # only after a couple of filler groups are queued so the
                # PE never parks on the tail exps.
                gens = []
                if pending_out is not None:
                    gens.append(emit_outproj(*pending_out))
                    pending_out = None
                if qb + 1 < NQB:
                    qa_next = []
                    gens.append(proj(qb + 1, xts_next, qa_next))
                passbs = [3, 2, 1, 0]
                # proj groups are ~4x the PE time of outproj groups;
                # scale the drain / pass_b cadence to the filler density
                drain_rnd = 3 if qb == 0 else (2 if not last else 6)
                rnd = 0
                while gens:
                    for g in list(gens):
                        if next(g, StopIteration) is StopIteration:
                            gens.remove(g)
                    rnd += 1
                    if rnd == drain_rnd:
                        drain_carry()
                        finish_all()
                    elif rnd >= drain_rnd + 2 and rnd % 2 == 0 and passbs:
                        pass_b(passbs.pop(0))
                if carry:
                    drain_carry()
                    finish_all()
                while passbs:
                    pass_b(passbs.pop(0))
                pending_out = (qb, ot_t)

            for _ in emit_outproj(*pending_out, dual=True):
                pass
    nc.finalize()
    return nc


_NC_CACHE = None


def _get_nc():
    global _NC_CACHE
    if _NC_CACHE is None:
        _NC_CACHE = _build()
    return _NC_CACHE


def kernel(x, Wq, bq, Wk, bk, Wv, bv, Wo, bo):
    x = np.asarray(x, dtype=np.float32)
    Wq, bq = np.asarray(Wq, np.float32), np.asarray(bq, np.float32)
    Wk, bk = np.asarray(Wk, np.float32), np.asarray(bk, np.float32)
    Wv, bv = np.asarray(Wv, np.float32), np.asarray(bv, np.float32)
    Wo, bo = np.asarray(Wo, np.float32), np.asarray(bo, np.float32)

    slopes = np.asarray(_slopes(H), dtype=np.float32)
    ar = np.arange(T, dtype=np.float32)

    # bias folding (device never sees biases):
    #   bv: softmax rows sum to 1 -> y += bv @ Wo, fold into bo.
    #   bk: contributes bk.(Wq x_q) + bq.bk to every score of column q --
    #       constant per query, softmax-invariant, dropped.
    #   bq: the surviving term bq.(Wk x_k) is per-key; precompute
    #       bqk[b, h, t] and ride it on aug row 3 / the hb table.
    bo_eff = bo + bv @ Wo
    have_bq = bool(np.any(bq))
    if have_bq:
        # [B, H, T] = per-head inner product of bq with the K projection
        kproj = x @ Wk  # [B, T, C]
        bqk = np.stack([
            np.stack([kproj[b, :, D * h:D * (h + 1)] @ bq[D * h:D * (h + 1)]
                      for h in range(H)], axis=0)
            for b in range(B)], axis=0)  # [B, H, T]
    else:
        bqk = np.zeros((B, H, T), np.float32)

    pp, ff = np.meshgrid(np.arange(128), np.arange(256), indexing="ij")
    stair_np = np.where(ff - 128 < pp, -3000.0, 0.0).astype(BF)
    ident_np = np.eye(128, dtype=np.float32).astype(BF)

    def panel(a, nchunk):
        # [nchunk*128, cols] -> [128, nchunk, cols] contraction panels
        return np.ascontiguousarray(
            a.reshape(nchunk, 128, a.shape[1]).transpose(1, 0, 2))

    xts = []
    for b in range(B):
        xts.append(panel(x[b].T.astype(BF), NCH))

    pr = np.arange(128, dtype=np.float32)
    in_maps = []
    for core in range(NCORES):
        b, g = divmod(core, HG)
        # strided head assignment: core g, slot j <-> global head 4j+g, so
        # each slot's ALiBi slope range is uniform across cores and the
        # (SPMD-shared) graph can window steep slots' attention
        heads = [HG * j + g for j in range(HG)]
        # ACT-bias table for slots 2,3: col = 64*(slot-2) + 16*qb + kt,
        # value[p] = slope * (128*kt + p - 512*qb) + bqk
        hb = np.zeros((128, 128), np.float32)
        for sl in (2, 3):
            h = heads[sl]
            s = slopes[h]
            for qbn in range(4):
                for kt in range(16):
                    col = 64 * (sl - 2) + 16 * qbn + kt
                    hb[:, col] = (s * (128.0 * kt + pr - 512.0 * qbn)
                                  + bqk[b, h, 128 * kt:128 * kt + 128])
        cols = np.concatenate([np.arange(D * h, D * (h + 1)) for h in heads])
        wqa = np.ascontiguousarray(Wq[:, cols])
        wka = np.ascontiguousarray(Wk[:, cols])
        wva = np.zeros((C, VW), np.float32)
        for j, h in enumerate(heads):
            wva[:, 65 * j:65 * j + D] = Wv[:, D * h:D * (h + 1)]
        woa = np.ascontiguousarray(Wo[cols, :])
        hk = np.empty((2, 3, T), np.float32)
        hq = np.empty((2, 3, T), np.float32)
        for j in range(2):
            h = heads[j]
            # K rows (k, s8, 8*bqk) pair with Q rows (s8, -q, 1):
            # S += s8*(k - q) + 8*bqk[k].  Integer k/q are exact on the
            # f32r grid and s8 rounds once, so the large terms cancel
            # exactly in the fp32 PSUM accumulator.
            s8 = 8.0 * slopes[h]
            hk[j, 0] = ar
            hk[j, 1] = s8
            hk[j, 2] = 8.0 * bqk[b, h]
            hq[j, 0] = s8
            hq[j, 1] = -ar
            hq[j, 2] = 1.0
        in_maps.append(dict(
            xt=xts[b],
            wq=panel(wqa.astype(BF), NCH),
            wk=panel(wka.astype(BF), NCH),
            wv=panel(wva.astype(BF), NCH),
            wo=panel(woa.astype(BF), 2),
            hka=hk, hqa=hq, stair=stair_np, ident=ident_np, hbias=hb))

    nc = _get_nc()
    res = run_bass_kernel_spmd(nc, in_maps, core_ids=list(range(NCORES)))

    out = np.empty((B, T, C), np.float32)
    for b in range(B):
        acc = res.results[4 * b]["y"].astype(np.float32).copy()
        for g in range(1, HG):
            acc += res.results[4 * b + g]["y"].astype(np.float32)
        out[b] = acc + bo_eff[None, :]
    return out


# revision 118
# speedup vs baseline: 1.0146x; 1.0146x over previous
"""ALiBi causal attention layer on 8 TRN2 NeuronCores.

Sharding: data parallel on batch (B=2) x tensor parallel on heads (16 -> 4
groups of 4).  Core c = 4*b + g computes, for batch element b, the STRIDED
head set {g, 4+g, 8+g, 12+g} end to end: QKV projections (column-sharded),
causal ALiBi attention, and the row-sharded output projection.  The host
sums the 4 partial outputs per batch element (the tensor-parallel
all-reduce) and adds the output bias.  The striding makes head slot j hold
global heads {4j..4j+3} on every core, so each slot's ALiBi slope range is
uniform and the SPMD-shared graph can window steep slots' attention: slot 0
(slopes >= 0.25) looks back only 56 positions, slot 1 (>= 0.0625) 224 --
skipped k-tiles contribute < 1e-11 to the softmax.

Device kernel (matmuls in bf16/f32r, fp32 PSUM accum):
  - x arrives host-transposed: xt [1024, 2048].  Projection biases never
    touch the device: bv folds into the host-side output bias (softmax
    rows sum to 1), bk's score contribution is constant per query column
    (softmax-invariant, dropped), and bq's surviving rank-1 term
    bq.(Wk x_k) rides a third ALiBi aug row (slots 0,1) / the per-k ACT
    bias table (slots 2,3) -- zeros when bq == 0.
  - K^T for steep slots 0,1 in per-head [128, 2048] f32r tiles: head data
    at its native partition parity, 3 aug rows (k, s8, 8*bqk) paired with
    Q rows (s8, -q, 1), remaining rows zeroed.  S^T = K_aug^T.T @ Q_aug,
    exp() on ACT with scale=1/8 (max-free softmax: scores bounded).
  - Shallow slots 2,3 get per-slot [128, T] bf16 K^T tiles at their
    native parity with the other half zeroed (the packed Q tile's
    other-slot rows multiply zeros); their ALiBi + bq term ride the
    exp's per-partition ACT bias (the per-q part cancels in the softmax).
  - Causality: k-tiles above the diagonal are skipped; diagonal tiles get
    -3000 on masked entries via a 128-col (tt=3: 256-col) staircase
    matmul accumulated pre-exp, so the exp underflows to 0.  Each slot's
    window also trims S/exp/PV columns beyond k_end + W.
  - V carries a ones column per head (den cols zero in the weights, a
    strided SBUF copy writes the ones), so PV yields O^T plus the softmax
    denominators.  All 4 heads' denominators stage at partitions
    {0,32,64,96} of one tile (legal matmul contraction bases): ONE
    reciprocal + ONE f32r cast per q-block serve the 4 broadcast matmuls
    (tile_position row-strips).  The divide writes ot rows directly --
    the DVE *can* remap partition blocks (out 64:128 from in 0:64).
  - Engine choreography: warm-up matmuls + a dummy exp run at t=0 under
    the initial DMA wait (PE p-state ramp + ACT table load off the
    critical path); every tensor loads with 1-2 DMA descriptors from
    host-interleaved [128, n, cols] panels; each head's trailing PVs lag
    the exp by 2 k-tiles and flush inside the NEXT emission site; each
    q-block's output projection is deferred into the NEXT block and
    interleaved round-robin with its projections (PSUM-evacuation lag
    drains under the dense proj groups) and with the pass_b chain; y
    evacuations alternate scalar/vector; PSUM po pool holds all 4 heads
    so PV never couples to pass_b.
"""
import math

import ml_dtypes
import numpy as np

BF = ml_dtypes.bfloat16

import concourse.bass as bass
import concourse.tile as tile
from concourse import mybir, bacc
from concourse.bass_utils import run_bass_kernel_spmd

F32 = mybir.dt.float32
F32R = mybir.dt.float32r
BF16 = mybir.dt.bfloat16

B, T, C, H = 2, 2048, 1024, 16
D = C // H            # 64 head dim
NCORES = 8
HG = 4                # heads per core
CG = HG * D           # 256 channels per core
VW = HG * (D + 1)     # 260: V with a ones column per head
QB = 512              # q block width
KTW = 128             # k tile width
NQB = T // QB         # 4
NKT = T // KTW        # 16
NCH = C // 128        # 8 contraction chunks


def _slopes(n):
    def p2(m):
        start = 2 ** (-(2 ** -(math.log2(m) - 3)))
        return [start * start**i for i in range(m)]
    if math.log2(n).is_integer():
        return p2(n)
    c = 2 ** math.floor(math.log2(n))
    return p2(c) + _slopes(2 * c)[0::2][: n - c]


def _build():
    nc = bacc.Bacc()
    # host pre-interleaves every matrix into [128, n*cols] panels (chunk c
    # of the contraction dim at columns [cols*c, cols*(c+1))) so each
    # tensor loads with a single contiguous DMA descriptor.
    xt = nc.declare_dram_parameter("xt", [128, NCH, T], BF16, isOutput=False)
    wq = nc.declare_dram_parameter("wq", [128, NCH, CG], BF16, isOutput=False)
    wk = nc.declare_dram_parameter("wk", [128, NCH, CG], BF16, isOutput=False)
    wv = nc.declare_dram_parameter("wv", [128, NCH, VW], BF16, isOutput=False)
    wo = nc.declare_dram_parameter("wo", [128, 2, C], BF16, isOutput=False)
    hka = nc.declare_dram_parameter("hka", [2, 3, T], F32R, isOutput=False)
    hqa = nc.declare_dram_parameter("hqa", [2, 3, T], F32R, isOutput=False)
    stair = nc.declare_dram_parameter("stair", [128, 256], BF16, isOutput=False)
    ident = nc.declare_dram_parameter("ident", [128, 128], BF16, isOutput=False)
    hbias = nc.declare_dram_parameter("hbias", [128, 128], F32, isOutput=False)
    y = nc.declare_dram_parameter("y", [T, C], BF16, isOutput=True)

    EXP = mybir.ActivationFunctionType.Exp
    CPY = mybir.ActivationFunctionType.Copy

    with tile.TileContext(nc) as tc, \
         nc.allow_low_precision(reason="fp32r/bf16 compute"):
        with tc.tile_pool(name="const", bufs=1) as cp, \
             tc.tile_pool(name="xtp", bufs=3) as xtp, \
             tc.tile_pool(name="qap", bufs=8) as qap, \
             tc.tile_pool(name="otp", bufs=4) as otp, \
             tc.tile_pool(name="ptp", bufs=10) as ptp, \
             tc.tile_pool(name="yp", bufs=2) as ypool, \
             tc.tile_pool(name="misc", bufs=2) as mp, \
             tc.tile_pool(name="ps", bufs=4, space="PSUM") as psp, \
             tc.tile_pool(name="po", bufs=4, space="PSUM") as pop:

            # ---- t=0: PE p-state warm-up + ACT table load, under the
            # initial DMA wait.  No data deps, so the scheduler runs these
            # immediately; ~3.4us of matmul activity un-throttles the PE
            # clock before the first real projection matmul issues.
            wtile = cp.tile([128, QB], BF16, tag="warm")
            nc.gpsimd.memset(wtile[:], 0.25)
            wps = psp.tile([128, QB], F32, tag="ps", name="warm_ps")
            for i in range(22):
                nc.tensor.matmul(wps[:], wtile[:, 0:128], wtile[:],
                                 start=True, stop=True, skip_group_check=True)
            wrd = mp.tile([1, 16], F32, tag="wrd")
            nc.vector.tensor_copy(wrd[:], wps[0:1, 0:16])
            scr = cp.tile([1, 16], F32, tag="scr")
            nc.scalar.activation(scr[:], wtile[0:1, 0:16], EXP,
                                 bias=0.0, scale=1.0)

            # ---- constants: weights, aug rows ----
            # DMA descriptor generation (~0.5us each) is spread across the
            # sync / scalar / gpsimd queues so the first projection's
            # inputs (wq + xt block 0) land as early as possible.
            wq_big = cp.tile([128, NCH, CG], BF16, tag="wqb")
            wk_big = cp.tile([128, NCH, CG], BF16, tag="wkb")
            wv_big = cp.tile([128, NCH, VW], BF16, tag="wvb")
            wo_big = cp.tile([128, 2, C], BF16, tag="wob")
            wq_sb = [wq_big[:, c, :] for c in range(NCH)]
            wk_sb = [wk_big[:, c, :] for c in range(NCH)]
            wv_sb = [wv_big[:, c, :] for c in range(NCH)]
            wo_sb = [wo_big[:, m, :] for m in range(2)]
            ones_fr = cp.tile([128, 128], F32R, tag="ones_fr")
            ones32 = cp.tile([128, 128], F32, tag="ones32")
            nc.vector.memset(ones32[:], 1.0)
            nc.vector.tensor_copy(ones_fr[:], ones32[:])
            zf = cp.tile([128, QB], F32, tag="zf")
            nc.vector.memset(zf[:], 0.0)
            vones = cp.tile([128, 4], F32, tag="vones")
            nc.vector.memset(vones[:], 1.0)
            # first-needed tensors split in two so the leading projection
            # matmuls start after half the transfer
            nc.scalar.dma_start(wq_big[:, 0:4, :], wq[:, 0:4, :])
            nc.scalar.dma_start(wq_big[:, 4:8, :], wq[:, 4:8, :])
            xta0 = xtp.tile([128, NCH, QB], BF16, tag="xt", name="xta0")
            for qtr in range(4):
                nc.sync.dma_start(xta0[:, 2 * qtr:2 * qtr + 2, :],
                                  xt[:, 2 * qtr:2 * qtr + 2, 0:QB])
            xts0 = [xta0[:, c, :] for c in range(NCH)]

            # causal-mask staircase: stair[p, f] = -3000 where f - 128 < p.
            # Accumulating I.T @ stair into the masked 128 (tt=3: 256)
            # columns of a diagonal S tile drives k > q scores to -3000
            # pre-exp, so the exp underflows to 0.
            stair_sb = cp.tile([128, 256], BF16, tag="stair")
            ident_sb = cp.tile([128, 128], BF16, tag="ident")
            hb_sb = cp.tile([128, 128], F32, tag="hb")
            nc.gpsimd.dma_start(stair_sb[:], stair[:])
            nc.gpsimd.dma_start(ident_sb[:], ident[:])
            nc.gpsimd.dma_start(hb_sb[:], hbias[:])

            # Slots 0,1 (steep ALiBi slopes): per-head K^T tiles with the
            # rank-3 aug-row ALiBi (+ bq rank-1 term).  Even head: data
            # rows 0:64, aug rows 64:67, zeros 67:128.  Odd head: aug 0:3,
            # zeros 3:64, data 64:128.  K aug = (k, s8, 8*bqk).
            # Slots 2,3 (shallow slopes): per-slot [128, T] bf16 K^T tiles
            # at the slot's native parity (slot2 rows 0:64, slot3 rows
            # 64:128) with the other half zeroed -- the packed Q tile's
            # other-slot rows multiply zeros.  Their ALiBi rides the exp
            # as a per-partition ACT bias (the per-q part cancels in the
            # softmax).
            ka2 = cp.tile([128, T], BF16, tag="ka2")
            ka3 = cp.tile([128, T], BF16, tag="ka3")
            ka = [cp.tile([128, T], F32R, tag=f"ka{h}", name=f"ka{h}") for h in range(2)]
            # ka zero-fills ride the startup-idle DVE (the scalar queue
            # must stay clear: it feeds the first exps); wv/wo descriptor
            # generation trails wq/xt0/wk so the startup-critical
            # transfers get the HBM bandwidth first.
            nc.scalar.dma_start(wk_big[:], wk[:])
            for h in range(2):
                arow = 64 if h % 2 == 0 else 0
                for blk in range(NQB):
                    sl = slice(QB * blk, QB * (blk + 1))
                    nc.scalar.mul(ka[h][arow:arow + 64, sl],
                                  zf[arow:arow + 64, :], 0.0)
                nc.gpsimd.dma_start(ka[h][arow:arow + 3, :], hka[h])
            nc.gpsimd.memset(ka2[64:128, :], 0.0)
            nc.gpsimd.memset(ka3[0:64, :], 0.0)
            nc.scalar.dma_start(wv_big[:], wv[:])
            nc.scalar.dma_start(wo_big[:], wo[:])

            v_sb = [cp.tile([128, VW], F32R, tag=f"v{t}", name=f"v{t}") for t in range(NKT)]

            # deferred final-PV + finish-head closures: flushed after the
            # next emission site has queued ready PE work, so the in-order
            # PE queue never parks on the tail exp of a head.
            carry = []

            def drain_carry():
                while carry:
                    carry.pop(0)()

            # ---- fused, software-pipelined per-block loop ----
            def fetch_x(qb):
                tsl = slice(QB * qb, QB * (qb + 1))
                xta = xtp.tile([128, NCH, QB], BF16, tag="xt",
                               name=f"xta{qb}")
                nc.sync.dma_start(xta[:], xt[:, :, tsl])
                return [xta[:, c, :] for c in range(NCH)]

            def proj(qb, xts, out_qa):
                """QKV projections for t-block qb (a generator: yields
                after each matmul group so the caller can interleave other
                emission between groups).  Appends the Q tiles to out_qa."""
                tsl = slice(QB * qb, QB * (qb + 1))

                qa_t = out_qa
                for h in range(2):
                    qat = qap.tile([128, QB], F32R, tag="qa",
                                   name=f"qa{qb}_{h}")
                    arow = 64 if h % 2 == 0 else 0
                    nc.vector.tensor_copy(qat[arow:arow + 64, :],
                                          zf[arow:arow + 64, :])
                    nc.scalar.dma_start(qat[arow:arow + 3, :],
                                        hqa[h][:, tsl])
                    qa_t.append(qat)
                q23 = qap.tile([128, QB], BF16, tag="q23",
                               name=f"q23_{qb}")
                qa_t.append(q23)

                for wi, (wsb, is_q) in enumerate(((wq_sb, True),
                                                  (wk_sb, False))):
                    for m in range(2):
                        ps = psp.tile([128, QB], F32, tag="ps")
                        for c in range(NCH):
                            nc.tensor.matmul(
                                ps[:], wsb[c][:, 128 * m:128 * (m + 1)],
                                xts[c][:], start=(c == 0), stop=(c == 7),
                                skip_group_check=True)
                        if m == 1:
                            # packed pair: slot2 rows 0:64, slot3 rows
                            # 64:128, exactly the proj PSUM layout
                            if is_q:
                                nc.vector.tensor_copy(q23[:], ps[:])
                            else:
                                nc.vector.tensor_copy(ka2[0:64, tsl],
                                                      ps[0:64, :])
                                nc.vector.tensor_copy(ka3[64:128, tsl],
                                                      ps[64:128, :])
                        else:
                            for j in range(2):
                                h = j
                                rows = slice(64 * j, 64 * j + 64)
                                if is_q:
                                    nc.vector.tensor_copy(qa_t[h][rows, :],
                                                          ps[rows, :])
                                else:
                                    nc.vector.tensor_copy(ka[h][rows, tsl],
                                                          ps[rows, :])
                        yield

                for tt in range(4):
                    kt = 4 * qb + tt
                    psv = psp.tile([128, QB], F32, tag="ps")
                    for c in range(NCH):
                        nc.tensor.matmul(
                            psv[:, 0:VW],
                            xts[c][:, 128 * tt:128 * (tt + 1)], wv_sb[c][:],
                            start=(c == 0), stop=(c == 7),
                            skip_group_check=True)
                    nc.vector.tensor_copy(v_sb[kt][:], psv[:, 0:VW])
                    # per-head softmax-denominator ones columns: one
                    # stride-65 copy covers all four
                    nc.vector.tensor_copy(v_sb[kt][:, D:VW:D + 1],
                                          vones[:])
                    yield

            qa_next = []
            for _ in proj(0, xts0, qa_next):
                pass
            pending_out = None
            for qb in range(NQB):
                qa_t = qa_next
                if qb + 1 < NQB:
                    xts_next = fetch_x(qb + 1)
                # attention for this q-block.  Pass A per head is the
                # PE-heavy S/exp/PV chain; pass B (recip -> broadcast
                # -> divide) for head h is emitted after head h+1's pass A
                # so the broadcast matmul never sits at the front of the PE
                # queue waiting on the DVE reciprocal.
                po_t = {}
                ot_t = [otp.tile([128, QB], BF16, tag="ot",
                                 name=f"ot_{qb}_{c}") for c in range(2)]

                # ALiBi windows per head slot: with the strided head
                # assignment, slot j holds global heads {4j..4j+3}; a tile
                # whose every (k, q) pair has slope*(k-q) <= -14 contributes
                # < 1e-4 relative attention mass (well under the 2e-2 rel-err
                # budget).  W_j = 14 / min-slope-in-slot.
                WIN = (56.0, 224.0, 897.0, 1e9)

                # Diagonal k-tile tt (tt = kt - 4*qb) only matters for q
                # columns >= 128*tt, so trim its S/exp/PV to [C_tt, 512).
                # tt=3 keeps 256 cols (f32r needs a >=256 moving dim); its
                # extra cols [256,384) are fully masked by the staircase.
                # The slot's ALiBi window trims the upper end too: columns
                # beyond k_end + W contribute < e^-14 attention mass.
                TRIM = ((0, QB), (128, 384), (256, 256), (256, 256))

                def trim(slot, kt):
                    tt = kt - 4 * qb
                    c0 = TRIM[tt][0] if tt >= 0 else 0
                    wmax = TRIM[tt][1] if tt >= 0 else QB
                    kend = 128 * (kt + 1) - 512 * qb  # block-relative
                    need = kend + WIN[slot] - c0
                    w = min(wmax, max(256, -(-int(need) // 128) * 128))
                    return c0, w

                den4 = mp.tile([128, QB], F32, tag="den4", bufs=2,
                               name=f"den4_{qb}")
                rc4 = mp.tile([128, QB], F32R, tag="rc4", bufs=2,
                              name=f"rc4_{qb}")

                # keep every den4 partition defined so the full-tile
                # reciprocal never reads uninitialized SBUF
                nc.gpsimd.memset(den4[:], 1.0)

                def finish_head(h, po):
                    # head h's denominator to partition 32h of the shared
                    # tile (a legal matmul contraction base); one
                    # reciprocal + one f32r cast then serve all 4 heads.
                    nc.vector.tensor_copy(den4[32 * h:32 * h + 1, :],
                                          po[D:D + 1, :])
                    po_t[h] = po

                def finish_all(tag=0):
                    # full-tile reciprocal/cast: DVE cost is free-dim
                    # dominated; rows whose head isn't finished yet hold
                    # the 1.0 memset (or a stale den) and are either never
                    # read or recomputed by a later call.
                    rc32 = mp.tile([128, QB], F32, tag="rc32", bufs=2,
                                   name=f"rc32_{qb}_{tag}")
                    nc.vector.reciprocal_approx_fast(rc32[:], den4[:])
                    nc.vector.tensor_copy(rc4[:], rc32[:])

                def pass_a(h):
                    # diagonal tiles go first so tile tt=0 opens the full
                    # [0,512) PV accumulation region and the head's tail is
                    # short-latency.  PV lags the S/exp chain by one k-tile
                    # so the PE never sits waiting on the ACT exp.
                    full = [kt for kt in range(4 * qb)
                            if 128 * kt > QB * qb - WIN[h] - 127]
                    kts = list(range(4 * qb, 4 * qb + 4)) + full
                    po = pop.tile([D + 1, QB], F32, tag="po",
                                  name=f"po_{qb}_{h}")
                    npv = [0]

                    def pv(pkt, pc0, pw, ppt, last):
                        nc.tensor.matmul(
                            po[:, pc0:pc0 + pw],
                            v_sb[pkt][:, 65 * h:65 * (h + 1)],
                            ppt[:, 0:pw], start=(npv[0] == 0), stop=last,
                            skip_group_check=True)
                        npv[0] += 1

                    pendq = []
                    for i, kt in enumerate(kts):
                        tt = kt - 4 * qb
                        c0, w = trim(h, kt)
                        pss = psp.tile([128, QB], F32, tag="ps")
                        nc.tensor.matmul(
                            pss[:, 0:w], ka[h][:, 128 * kt:128 * (kt + 1)],
                            qa_t[h][:, c0:c0 + w], start=True, stop=(tt < 0),
                            skip_group_check=True)
                        if tt >= 0:
                            # masked (k > q) entries get -3000 pre-exp; only
                            # the 128 cols crossing the diagonal (tt=3: the
                            # 256 cols at/below it) need the staircase
                            soff, sw = (128, 128) if tt < 3 else (0, 256)
                            nc.tensor.matmul(
                                pss[:, 0:sw], ident_sb[:],
                                stair_sb[:, soff:soff + sw], start=False,
                                stop=True, skip_group_check=True)
                        pt = ptp.tile([128, QB], F32R, tag="pt")
                        nc.scalar.activation(pt[:, 0:w], pss[:, 0:w], EXP,
                                             bias=0.0, scale=0.125)
                        if i == 1:
                            drain_carry()
                        # PV lags the exp by two k-tiles: ~640ns of S work
                        # separates each PV from its exp's completion
                        pendq.append((kt, c0, w, pt))
                        if len(pendq) > 3:
                            pv(*pendq.pop(0), last=False)

                    def fin(pendq=pendq):
                        while len(pendq) > 1:
                            pv(*pendq.pop(0), last=False)
                        pv(*pendq.pop(0), last=True)
                        finish_head(h, po)
                    carry.append(fin)

                def pass_a23():
                    # slots 2,3 share one packed K/Q tile; common k-tiles
                    # issue as two concurrent row-tiled S matmuls.  PV lags
                    # by one k-tile so the PE never waits on the exp.
                    full2 = [kt for kt in range(4 * qb)
                             if 128 * kt > QB * qb - WIN[2] - 127]
                    kts = list(range(4 * qb, 4 * qb + 4)) + list(range(4 * qb))
                    po2 = pop.tile([D + 1, QB], F32, tag="po",
                                   name=f"po_{qb}_2")
                    po3 = pop.tile([D + 1, QB], F32, tag="po",
                                   name=f"po_{qb}_3")
                    n2 = 4 + len(full2)
                    n3 = len(kts)
                    i2 = [0]
                    i3 = [0]

                    def pv_flush(pend):
                        pkt, pc0, pw2, pw3, pt2, pt3 = pend
                        if pt2 is not None:
                            nc.tensor.matmul(
                                po2[:, pc0:pc0 + pw2],
                                v_sb[pkt][:, 65 * 2:65 * 3], pt2[:, 0:pw2],
                                start=(i2[0] == 0), stop=(i2[0] == n2 - 1),
                                skip_group_check=True)
                            i2[0] += 1
                        nc.tensor.matmul(
                            po3[:, pc0:pc0 + pw3],
                            v_sb[pkt][:, 65 * 3:65 * 4], pt3[:, 0:pw3],
                            start=(i3[0] == 0), stop=(i3[0] == n3 - 1),
                            skip_group_check=True)
                        i3[0] += 1

                    pendq = []
                    for i, kt in enumerate(kts):
                        tt = kt - 4 * qb
                        c0, w2 = trim(2, kt)
                        c0, w3 = trim(3, kt)
                        ktsl = slice(128 * kt, 128 * (kt + 1))
                        has2 = tt >= 0 or kt in full2
                        if has2:
                            pss2 = psp.tile([128, QB], F32, tag="ps")
                            nc.tensor.matmul(
                                pss2[:, 0:w2], ka2[:, ktsl],
                                qa_t[2][:, c0:c0 + w2], start=True,
                                stop=(tt < 0), skip_group_check=True)
                        pss3 = psp.tile([128, QB], F32, tag="ps")
                        nc.tensor.matmul(
                            pss3[:, 0:w3], ka3[:, ktsl],
                            qa_t[2][:, c0:c0 + w3], start=True,
                            stop=(tt < 0), skip_group_check=True)
                        if tt >= 0:
                            soff, sw = (128, 128) if tt < 3 else (0, 256)
                            nc.tensor.matmul(
                                pss2[:, 0:sw], ident_sb[:],
                                stair_sb[:, soff:soff + sw], start=False,
                                stop=True, skip_group_check=True)
                            nc.tensor.matmul(
                                pss3[:, 0:sw], ident_sb[:],
                                stair_sb[:, soff:soff + sw], start=False,
                                stop=True, skip_group_check=True)
                        pt2 = None
                        if has2:
                            bcol = 16 * qb + kt
                            pt2 = ptp.tile([128, QB], F32R, tag="pt")
                            nc.scalar.activation(
                                pt2[:, 0:w2], pss2[:, 0:w2], EXP,
                                bias=hb_sb[:, bcol:bcol + 1], scale=0.125)
                        bcol = 64 + 16 * qb + kt
                        pt3 = ptp.tile([128, QB], F32R, tag="pt")
                        nc.scalar.activation(
                            pt3[:, 0:w3], pss3[:, 0:w3], EXP,
                            bias=hb_sb[:, bcol:bcol + 1], scale=0.125)
                        if i == 1:
                            drain_carry()
                        pendq.append((kt, c0, w2, w3, pt2, pt3))
                        if len(pendq) > 4:
                            pv_flush(pendq.pop(0))

                    def fin(pendq=pendq):
                        while pendq:
                            pv_flush(pendq.pop(0))
                        finish_head(3, po3)
                        finish_head(2, po2)
                    carry.append(fin)

                def pass_b(h):
                    po = po_t.pop(h)
                    pb = psp.tile([D, QB], F32, tag="ps",
                                  name=f"pb_{qb}_{h}")
                    nc.tensor.matmul(pb[:], ones_fr[32 * h:32 * h + 1, 0:D],
                                     rc4[32 * h:32 * h + 1, :],
                                     start=True, stop=True,
                                     skip_group_check=True,
                                     tile_position=(32 * h, 0))
                    bc = mp.tile([D, QB], F32, tag="bc", bufs=4,
                                 name=f"bc_{qb}_{h}")
                    nc.vector.tensor_copy(bc[:], pb[:])
                    pair = ot_t[h // 2]
                    if h % 2 == 0:
                        nc.vector.tensor_tensor(pair[0:D, :], po[0:D, :],
                                                bc[:],
                                                op=mybir.AluOpType.mult)
                    else:
                        # odd head's O^T lands at partitions 0:64; DVE
                        # cannot shift multi-partition blocks, so divide
                        # into a temp then DMA it into rows 64:128
                        nc.vector.tensor_tensor(pair[D:2 * D, :],
                                                po[0:D, :], bc[:],
                                                op=mybir.AluOpType.mult)

                def emit_outproj(oqb, ot_pair, dual=False):
                    """Generator: yields after each half-tile so the
                    caller can interleave projection groups between the
                    PSUM-evacuation-paced output-projection groups.  With
                    dual=True each evacuation is split across the scalar
                    AND vector engines (for the final block, which has no
                    filler to hide the evacuation lag behind)."""
                    for tt in range(4):
                        t = 4 * oqb + tt
                        fsl = slice(128 * tt, 128 * (tt + 1))
                        ysb = ypool.tile([128, C], BF16, tag="y",
                                         name=f"y_{oqb}_{tt}")
                        for half in range(2):
                            hsl = slice(QB * half, QB * (half + 1))
                            py = psp.tile([128, QB], F32, tag="ps")
                            for c in (1, 0):
                                nc.tensor.matmul(
                                    py[:], ot_pair[c][:, fsl],
                                    wo_sb[c][:, hsl],
                                    start=(c == 1), stop=(c == 0),
                                    skip_group_check=True)
                            # alternate the PSUM evacuation between the
                            # scalar and vector engines: neither engine
                            # alone can keep up with the PE here
                            if dual:
                                h0 = slice(QB * half, QB * half + 256)
                                h1 = slice(QB * half + 256, QB * (half + 1))
                                nc.scalar.activation(ysb[:, h0],
                                                     py[:, 0:256], CPY)
                                nc.vector.tensor_copy(ysb[:, h1],
                                                      py[:, 256:QB])
                            elif (2 * tt + half) % 2 == 0:
                                nc.scalar.activation(ysb[:, hsl], py[:], CPY)
                            else:
                                nc.vector.tensor_copy(ysb[:, hsl], py[:])
                            # the final block's writebacks alternate queues
                            # so their descriptor generation parallelizes
                            # (it's the kernel's tail)
                            dq = (nc.sync if dual and half == 0 else
                                  nc.gpsimd)
                            dq.dma_start(y[128 * t:128 * (t + 1), hsl],
                                         ysb[:, hsl])
                            yield

                # Slots 2,3 (largest k-tile count) first.  All four heads'
                # denominators land in den4, then one reciprocal serves the
                # four pass_b's.  The PREVIOUS q-block's output projection
                # and the NEXT block's projections are emitted under the
                # pass_b chains: the scheduler fills every PE stall with
                # that ready work.
                pass_a23()
                pass_a(1)
                last = qb + 1 == NQB
                pass_a(0)
                # interleave three streams over the rest of the block:
                # the previous block's output projection (PSUM-evacuation
                # paced), the next block's input projections (dense PE
                # filler), and this block's pass_b chain (DVE-latency
                # bound) -- plus the deferred attention-tail PVs, drained
                # only after a couple of filler groups are queued so the
                # PE never parks on the tail exps.
                gens = []
                if pending_out is not None:
                    gens.append(emit_outproj(*pending_out))
                    pending_out = None
                if qb + 1 < NQB:
                    qa_next = []
                    gens.append(proj(qb + 1, xts_next, qa_next))
                passbs = [3, 2, 1, 0]
                # proj groups are ~4x the PE time of outproj groups;
                # scale the drain / pass_b cadence to the filler density
                drain_rnd = 3 if qb == 0 else (2 if not last else 6)
                rnd = 0
                while gens:
                    for g in list(gens):
                        if next(g, StopIteration) is StopIteration:
                            gens.remove(g)
                    rnd += 1
                    if rnd == drain_rnd:
                        drain_carry()
                        finish_all()
                    elif rnd >= drain_rnd + 2 and rnd % 2 == 0 and passbs:
                        pass_b(passbs.pop(0))
                if carry:
                    drain_carry()
                    finish_all()
                while passbs:
                    pass_b(passbs.pop(0))
                pending_out = (qb, ot_t)

            for _ in emit_outproj(*pending_out, dual=True):
                pass
    nc.finalize()
    return nc


_NC_CACHE = None


def _get_nc():
    global _NC_CACHE
    if _NC_CACHE is None:
        _NC_CACHE = _build()
    return _NC_CACHE


def kernel(x, Wq, bq, Wk, bk, Wv, bv, Wo, bo):
    x = np.asarray(x, dtype=np.float32)
    Wq, bq = np.asarray(Wq, np.float32), np.asarray(bq, np.float32)
    Wk, bk = np.asarray(Wk, np.float32), np.asarray(bk, np.float32)
    Wv, bv = np.asarray(Wv, np.float32), np.asarray(bv, np.float32)
    Wo, bo = np.asarray(Wo, np.float32), np.asarray(bo, np.float32)

    slopes = np.asarray(_slopes(H), dtype=np.float32)
    ar = np.arange(T, dtype=np.float32)

    # bias folding (device never sees biases):
    #   bv: softmax rows sum to 1 -> y += bv @ Wo, fold into bo.
    #   bk: contributes bk.(Wq x_q) + bq.bk to every score of column q --
    #       constant per query, softmax-invariant, dropped.
    #   bq: the surviving term bq.(Wk x_k) is per-key; precompute
    #       bqk[b, h, t] and ride it on aug row 3 / the hb table.
    bo_eff = bo + bv @ Wo
    have_bq = bool(np.any(bq))
    if have_bq:
        # [B, H, T] = per-head inner product of bq with the K projection
        kproj = x @ Wk  # [B, T, C]
        bqk = np.stack([
            np.stack([kproj[b, :, D * h:D * (h + 1)] @ bq[D * h:D * (h + 1)]
                      for h in range(H)], axis=0)
            for b in range(B)], axis=0)  # [B, H, T]
    else:
        bqk = np.zeros((B, H, T), np.float32)

    pp, ff = np.meshgrid(np.arange(128), np.arange(256), indexing="ij")
    stair_np = np.where(ff - 128 < pp, -3000.0, 0.0).astype(BF)
    ident_np = np.eye(128, dtype=np.float32).astype(BF)

    def panel(a, nchunk):
        # [nchunk*128, cols] -> [128, nchunk, cols] contraction panels
        return np.ascontiguousarray(
            a.reshape(nchunk, 128, a.shape[1]).transpose(1, 0, 2))

    xts = []
    for b in range(B):
        xts.append(panel(x[b].T.astype(BF), NCH))

    pr = np.arange(128, dtype=np.float32)
    in_maps = []
    for core in range(NCORES):
        b, g = divmod(core, HG)
        # strided head assignment: core g, slot j <-> global head 4j+g, so
        # each slot's ALiBi slope range is uniform across cores and the
        # (SPMD-shared) graph can window steep slots' attention
        heads = [HG * j + g for j in range(HG)]
        # ACT-bias table for slots 2,3: col = 64*(slot-2) + 16*qb + kt,
        # value[p] = slope * (128*kt + p - 512*qb) + bqk
        hb = np.zeros((128, 128), np.float32)
        for sl in (2, 3):
            h = heads[sl]
            s = slopes[h]
            for qbn in range(4):
                for kt in range(16):
                    col = 64 * (sl - 2) + 16 * qbn + kt
                    hb[:, col] = (s * (128.0 * kt + pr - 512.0 * qbn)
                                  + bqk[b, h, 128 * kt:128 * kt + 128])
        cols = np.concatenate([np.arange(D * h, D * (h + 1)) for h in heads])
        wqa = np.ascontiguousarray(Wq[:, cols])
        wka = np.ascontiguousarray(Wk[:, cols])
        wva = np.zeros((C, VW), np.float32)
        for j, h in enumerate(heads):
            wva[:, 65 * j:65 * j + D] = Wv[:, D * h:D * (h + 1)]
        woa = np.ascontiguousarray(Wo[cols, :])
        hk = np.empty((2, 3, T), np.float32)
        hq = np.empty((2, 3, T), np.float32)
        for j in range(2):
            h = heads[j]
            # K rows (k, s8, 8*bqk) pair with Q rows (s8, -q, 1):
            # S += s8*(k - q) + 8*bqk[k].  Integer k/q are exact on the
            # f32r grid and s8 rounds once, so the large terms cancel
            # exactly in the fp32 PSUM accumulator.
            s8 = 8.0 * slopes[h]
            hk[j, 0] = ar
            hk[j, 1] = s8
            hk[j, 2] = 8.0 * bqk[b, h]
            hq[j, 0] = s8
            hq[j, 1] = -ar
            hq[j, 2] = 1.0
        in_maps.append(dict(
            xt=xts[b],
            wq=panel(wqa.astype(BF), NCH),
            wk=panel(wka.astype(BF), NCH),
            wv=panel(wva.astype(BF), NCH),
            wo=panel(woa.astype(BF), 2),
            hka=hk, hqa=hq, stair=stair_np, ident=ident_np, hbias=hb))

    nc = _get_nc()
    res = run_bass_kernel_spmd(nc, in_maps, core_ids=list(range(NCORES)))

    out = np.empty((B, T, C), np.float32)
    for b in range(B):
        acc = res.results[4 * b]["y"].astype(np.float32).copy()
        for g in range(1, HG):
            acc += res.results[4 * b + g]["y"].astype(np.float32)
        out[b] = acc + bo_eff[None, :]
    return out
